# revision 26
# baseline (speedup 1.0000x reference)
"""Multi-head self-attention Trainium2 kernel (8 NeuronCores), v3.

Sharding: 8 cores = 4 batches x 2 head-groups (8 heads each).
Core c handles batch b=c//2, heads [g*8, (g+1)*8) where g=c%2.
Each core computes a partial output (its heads' contribution to the
output projection); the host sums the two partials per batch and adds bo.

v3 design (vs the 836us v1 baseline):
- bf16 stationary operands everywhere (fast, overlapped LDWEIGHTS vs the
  serial in-matmul 4-byte load fp32r pays); x/Q/K/V paths bf16.
- B-phase software pipeline per (pair, jt) stage: scores for both pair
  halves (row-packed, concurrent on the PE's 64-row groups) land in one
  [128,1024] PSUM tile; ONE exp instruction per stage (halves the ACT
  per-instruction overhead); PV lags and consumes jt-PAIRS.
- PV runs in fp8e4m3 with DoubleRow perf mode: contraction = 256 keys
  (two interleaved 128-key planes), halving PV matmul count. exp writes
  fp8 directly. The ones-column trick still emits the softmax
  normalizer Z as ct row 64 (fp8 1.0 is exact; accumulation is f32).
- Normalizer: ct evacuates to SBUF immediately (fast PSUM bank release,
  ct pool = 2 banks), then 1/Z via fast custom-DVE reciprocal (input
  must sit at partition 0 - custom DVE/GpSimd ops read the wrong
  partition otherwise), GpSimd partition-broadcast, in-place multiply.
- Output projection packs head pairs into PE row groups; even/odd
  accumulate in SEPARATE PSUM banks (concurrent row-tiled matmuls into
  one bank abort the device) and the DVE evacuation adds the halves.
  C work interleaves into the next qb's attention stages.
"""

import numpy as np

import concourse.bass as bass
import concourse.tile as tile
from concourse import bacc, mybir
from contextlib import ExitStack

P = 128
D = 1024
HD = 512  # head dims per core (8 heads x 64)
NPAIR = 4
NH = 8
F32 = mybir.dt.float32
BF16 = mybir.dt.bfloat16
FP8 = mybir.dt.float8e4
EXPF = mybir.ActivationFunctionType.Exp
MULT = mybir.AluOpType.mult
ADD = mybir.AluOpType.add
DR = mybir.MatmulPerfMode.DoubleRow


def build_nc(S=2048, pv_fp8=False):
    NKT = D // P          # 8 k-tiles over model dim
    NJT = S // P          # 16 key tiles
    NJP = NJT // 2        # 8 key-tile pairs (DoubleRow PV)
    MSEG = 512
    NMSEG = S // MSEG     # 4
    QB = 512
    NQB = S // QB         # 4
    VW = 66 if pv_fp8 else 65  # head block width in VS (65 used + fp8 pad)

    nc = bacc.Bacc("TRN2", target_bir_lowering=False, debug=False)
    xT = nc.dram_tensor("xT", [D, S], BF16, kind="ExternalInput").ap()
    wq = nc.dram_tensor("wq", [D, HD], BF16, kind="ExternalInput").ap()
    wk = nc.dram_tensor("wk", [D, HD], BF16, kind="ExternalInput").ap()
    wv = nc.dram_tensor("wv", [D, HD], BF16, kind="ExternalInput").ap()
    wo = nc.dram_tensor("wo", [HD, D], BF16, kind="ExternalInput").ap()
    out = nc.dram_tensor("out", [S, D], F32, kind="ExternalOutput").ap()

    with tile.TileContext(nc) as tc:
        with ExitStack() as persist:
            data_pool = persist.enter_context(tc.tile_pool(name="data", bufs=1))

            QT = [data_pool.tile([P, S], BF16, tag=f"qt{p}", name=f"qt{p}")
                  for p in range(NPAIR)]
            KT = [data_pool.tile([P, S], BF16, tag=f"kt{p}", name=f"kt{p}")
                  for p in range(NPAIR)]
            if pv_fp8:
                # VS2[jp]: [128 tokens, 2 key planes, 8 heads x (64+ones+pad)]
                # plane t holds key tile 2*jp+t; col 64 of each head block is
                # 1.0 (memset) so PV emits Z as ct row 64.
                VS2 = [data_pool.tile([P, 2, NH * VW], FP8, tag=f"vs{j}",
                                      name=f"vs{j}") for j in range(NJP)]
                for j in range(NJP):
                    nc.vector.memset(VS2[j][:], 1.0)
            else:
                VS = [data_pool.tile([P, NH * VW], BF16, tag=f"vs{j}",
                                     name=f"vs{j}") for j in range(NJT)]
                for j in range(NJT):
                    nc.vector.memset(VS[j][:], 1.0)

            # bias keeps exp within fp8e4m3 range (max 448); softmax is
            # shift-invariant so Z absorbs the e^-1.5 factor.
            expb = data_pool.tile([P, 1], F32, tag="expb", name="expb")
            nc.vector.memset(expb[:], -1.5)

            # wo per head, all at base partition 0 (C runs unpacked).
            wo_h = []
            for h in range(NH):
                t = data_pool.tile([64, D], BF16, tag=f"wo{h}", name=f"wo{h}")
                nc.sync.dma_start(t[:], wo[h * 64:(h + 1) * 64, :])
                wo_h.append(t)

            # ---------------- Phase A: projections ----------------
            with ExitStack() as es_a:
                w_pool = es_a.enter_context(tc.tile_pool(name="wpool", bufs=1))
                chunk_pool = es_a.enter_context(tc.tile_pool(name="chunks", bufs=6))

                wq_t = w_pool.tile([P, NKT, HD], BF16, tag="wq", name="wq_t")
                nc.sync.dma_start(wq_t[:], wq.rearrange("(kt p) n -> p kt n", p=P))
                wk_t = w_pool.tile([P, NKT, HD], BF16, tag="wk", name="wk_t")
                nc.sync.dma_start(wk_t[:], wk.rearrange("(kt p) n -> p kt n", p=P))
                wv_t = w_pool.tile([P, NKT, HD], BF16, tag="wv", name="wv_t")
                nc.sync.dma_start(wv_t[:], wv.rearrange("(kt p) n -> p kt n", p=P))

                # --- A1: QT / KT (8 PSUM accumulators: (q|k) x 4 pairs) ---
                with tc.tile_pool(name="qkps", bufs=8, space="PSUM") as qk_pool:
                    for mseg in range(NMSEG):
                        accs = [qk_pool.tile([P, MSEG], F32, tag="qk", name="qkacc")
                                for _ in range(8)]
                        for kt in range(NKT):
                            xc = chunk_pool.tile([P, MSEG], BF16, tag="xc", name="xc")
                            nc.sync.dma_start(
                                xc[:],
                                xT[kt * P:(kt + 1) * P, mseg * MSEG:(mseg + 1) * MSEG])
                            for p in range(NPAIR):
                                for ti, wt in ((0, wq_t), (1, wk_t)):
                                    nc.tensor.matmul(
                                        accs[p * 2 + ti][:],
                                        lhsT=wt[:, kt, p * P:(p + 1) * P],
                                        rhs=xc[:],
                                        start=(kt == 0), stop=(kt == NKT - 1))
                        for p in range(NPAIR):
                            nc.vector.tensor_copy(
                                QT[p][:, mseg * MSEG:(mseg + 1) * MSEG], accs[p * 2][:])
                            nc.vector.tensor_copy(
                                KT[p][:, mseg * MSEG:(mseg + 1) * MSEG], accs[p * 2 + 1][:])

                # --- A2: V (natural layout, 4 j-tiles per mseg) ---
                with tc.tile_pool(name="vps", bufs=8, space="PSUM") as v_pool:
                    for mseg in range(NMSEG):
                        vaccs = [v_pool.tile([P, HD], F32, tag="v", name="vacc")
                                 for _ in range(4)]
                        for kt in range(NKT):
                            xc = chunk_pool.tile([P, MSEG], BF16, tag="xc", name="xc")
                            nc.sync.dma_start(
                                xc[:],
                                xT[kt * P:(kt + 1) * P, mseg * MSEG:(mseg + 1) * MSEG])
                            for i in range(4):
                                nc.tensor.matmul(
                                    vaccs[i][:],
                                    lhsT=xc[:, i * P:(i + 1) * P],
                                    rhs=wv_t[:, kt, :],
                                    start=(kt == 0), stop=(kt == NKT - 1))
                        for i in range(4):
                            jt = mseg * 4 + i
                            if pv_fp8:
                                vsv = VS2[jt // 2][:, jt % 2, :].rearrange(
                                    "p (h c) -> p h c", c=VW)
                            else:
                                vsv = VS[jt].rearrange("p (h c) -> p h c", c=VW)
                            nc.vector.tensor_copy(vsv[:, :, 0:64], vaccs[i][:])

            # ---------------- Phases B + C: attention + projection ----------------
            with ExitStack() as es_b:
                st_ps = es_b.enter_context(tc.tile_pool(name="stps", bufs=2, space="PSUM"))
                ct_ps = es_b.enter_context(tc.tile_pool(name="ctps", bufs=3, space="PSUM"))
                po_ps = es_b.enter_context(tc.tile_pool(name="pops", bufs=1, space="PSUM"))
                pt_pool = es_b.enter_context(tc.tile_pool(name="ptpool", bufs=3))
                nrm_pool = es_b.enter_context(tc.tile_pool(name="nrm", bufs=2))
                cth_pool = es_b.enter_context(tc.tile_pool(name="cthpool", bufs=2))
                posb_pool = es_b.enter_context(tc.tile_pool(name="posb", bufs=3))

                def emit_scores(qb, p, jt):
                    """Both pair halves into one [128,1024] st tile (the two
                    row-group matmuls run concurrently, separate banks)."""
                    qs = slice(qb * QB, (qb + 1) * QB)
                    ks = slice(jt * P, (jt + 1) * P)
                    st = st_ps.tile([P, 2 * QB], F32, tag="st", name="st")
                    nc.tensor.matmul(st[:, 0:QB], lhsT=KT[p][0:64, ks],
                                     rhs=QT[p][0:64, qs], start=True, stop=True)
                    nc.tensor.matmul(st[:, QB:2 * QB], lhsT=KT[p][64:128, ks],
                                     rhs=QT[p][64:128, qs], start=True, stop=True)
                    return st

                def emit_pv_fp8(p, jp, ptp, cts):
                    h0, h1 = 2 * p, 2 * p + 1
                    nc.tensor.matmul(cts[0][0:65, :],
                                     lhsT=VS2[jp][:, :, h0 * VW:h0 * VW + 65],
                                     rhs=ptp[:, :, 0:QB],
                                     start=(jp == 0), stop=(jp == NJP - 1),
                                     perf_mode=DR)
                    nc.tensor.matmul(cts[1][0:65, :],
                                     lhsT=VS2[jp][:, :, h1 * VW:h1 * VW + 65],
                                     rhs=ptp[:, :, QB:2 * QB],
                                     start=(jp == 0), stop=(jp == NJP - 1),
                                     perf_mode=DR)

                def emit_pv_bf16(p, jt, ptp, cts):
                    h0, h1 = 2 * p, 2 * p + 1
                    nc.tensor.matmul(cts[0][0:65, :],
                                     lhsT=VS[jt][:, h0 * VW:h0 * VW + 65],
                                     rhs=ptp[:, 0:QB],
                                     start=(jt == 0), stop=(jt == NJT - 1))
                    nc.tensor.matmul(cts[1][0:65, :],
                                     lhsT=VS[jt][:, h1 * VW:h1 * VW + 65],
                                     rhs=ptp[:, QB:2 * QB],
                                     start=(jt == 0), stop=(jt == NJT - 1))

                def emit_norm(p, cts):
                    """Evacuate ct to SBUF right away (the 2 evac copies are
                    all that gates the ct bank reuse), then normalize
                    off-bank: 1/Z at partition 0 (custom DVE and GpSimd ops
                    read/write the wrong partition unless their APs start at
                    partition 0), GpSimd broadcast, in-place multiply. Only
                    the C phase consumes the result - a full qb later."""
                    cthE = cth_pool.tile([64, QB], BF16, tag=f"ce{p}", name=f"ce{p}")
                    cthO = cth_pool.tile([64, QB], BF16, tag=f"co{p}", name=f"co{p}")
                    nc.vector.tensor_copy(cthE[:], cts[0][0:64, :])
                    nc.vector.tensor_copy(cthO[:], cts[1][0:64, :])
                    zst = nrm_pool.tile([1, 2 * QB], F32, tag="zst", name="zst")
                    nc.vector.tensor_copy(zst[0:1, 0:QB], cts[0][64:65, :])
                    nc.vector.tensor_copy(zst[0:1, QB:2 * QB], cts[1][64:65, :])
                    zr = nrm_pool.tile([1, 2 * QB], F32, tag="zr", name="zr")
                    nc.vector.reciprocal_approx_fast(zr[0:1, :], zst[0:1, :])
                    zrb0 = nrm_pool.tile([64, QB], F32, tag="zrb0", name="zrb0")
                    zrb1 = nrm_pool.tile([64, QB], F32, tag="zrb1", name="zrb1")
                    nc.gpsimd.partition_broadcast(zrb0[:], zr[0:1, 0:QB])
                    nc.gpsimd.partition_broadcast(zrb1[:], zr[0:1, QB:2 * QB])
                    nc.vector.tensor_tensor(cthE[:], cthE[:], zrb0[:], MULT)
                    nc.vector.tensor_tensor(cthO[:], cthO[:], zrb1[:], MULT)
                    return (cthE, cthO)

                def emit_c_group(qb, gi, cth):
                    """One output tile: 8 serial base-0 head matmuls into a
                    single PSUM bank (concurrent row-tiled matmuls into one
                    bank abort the device; serial base-0 keeps it simple)."""
                    mtl, half = divmod(gi, 2)
                    mt = qb * 4 + mtl
                    ms = slice(mtl * P, (mtl + 1) * P)
                    hs = slice(half * 512, (half + 1) * 512)
                    po = po_ps.tile([P, 512], F32, tag="po", name="po")
                    for h in range(NH):
                        nc.tensor.matmul(po[:], lhsT=cth[h // 2][h % 2][:, ms],
                                         rhs=wo_h[h][:, hs],
                                         start=(h == 0), stop=(h == NH - 1))
                    po_sb = posb_pool.tile([P, 512], F32, tag="posb", name="po_sb")
                    nc.vector.tensor_copy(po_sb[:], po[:])
                    nc.sync.dma_start(out[mt * P:(mt + 1) * P, hs], po_sb[:])

                def emit_pv_group(item, cth_cur):
                    pp, pj, pptp, pcts = item
                    if pv_fp8:
                        emit_pv_fp8(pp, pj, pptp, pcts)
                        if pj == NJP - 1:
                            cth_cur[pp] = emit_norm(pp, pcts)
                    else:
                        emit_pv_bf16(pp, pj, pptp, pcts)
                        if pj == NJT - 1:
                            cth_cur[pp] = emit_norm(pp, pcts)

                prev_cth = None
                for qb in range(NQB):
                    cth_cur = [None] * NPAIR
                    cts = None
                    ptp = None
                    pv_queue = []  # (p, jp_or_jt, ptp, cts) ready for PV
                    sidx = 0
                    for p in range(NPAIR):
                        for jt in range(NJT):
                            if jt == 0:
                                cts = [ct_ps.tile([P, QB], F32, tag="ct", name="cte"),
                                       ct_ps.tile([P, QB], F32, tag="ct", name="cto")]
                            st = emit_scores(qb, p, jt)
                            if pv_fp8:
                                if jt % 2 == 0:
                                    ptp = pt_pool.tile([P, 2, 2 * QB], FP8,
                                                       tag="pt", name="ptp")
                                nc.scalar.activation(ptp[:, jt % 2, :], st[:],
                                                     EXPF, scale=0.125,
                                                     bias=expb[:])
                                if jt % 2 == 1:
                                    pv_queue.append((p, jt // 2, ptp, cts))
                            else:
                                ptp = pt_pool.tile([P, 2 * QB], BF16,
                                                   tag="pt", name="ptp")
                                nc.scalar.activation(ptp[:], st[:],
                                                     EXPF, scale=0.125)
                                pv_queue.append((p, jt, ptp, cts))
                            # PV for the previous ready group (one-stage lag)
                            if len(pv_queue) > 1:
                                emit_pv_group(pv_queue.pop(0), cth_cur)
                            if prev_cth is not None and sidx % 8 == 7:
                                emit_c_group(qb - 1, sidx // 8, prev_cth)
                            sidx += 1
                    while pv_queue:
                        emit_pv_group(pv_queue.pop(0), cth_cur)
                    prev_cth = cth_cur
                # final qb's projection
                for gi in range(8):
                    emit_c_group(NQB - 1, gi, prev_cth)
    nc.compile()
    return nc


_NC_CACHE = {}


def _get_nc(S=2048):
    if S not in _NC_CACHE:
        _NC_CACHE[S] = build_nc(S)
    return _NC_CACHE[S]


def make_in_maps(x, Wq, Wk, Wv, Wo):
    import ml_dtypes
    bf16 = ml_dtypes.bfloat16
    in_maps = []
    for c in range(8):
        b, g = divmod(c, 2)
        cols = slice(g * HD, (g + 1) * HD)
        in_maps.append({
            "xT": np.ascontiguousarray(x[b].T).astype(bf16),
            "wq": np.ascontiguousarray(Wq[:, cols]).astype(bf16),
            "wk": np.ascontiguousarray(Wk[:, cols]).astype(bf16),
            "wv": np.ascontiguousarray(Wv[:, cols]).astype(bf16),
            "wo": np.ascontiguousarray(Wo[cols, :]).astype(bf16),
        })
    return in_maps


def kernel(x, Wq, Wk, Wv, Wo, bo):
    from concourse.bass_utils import run_bass_kernel_spmd

    x = np.asarray(x, dtype=np.float32)
    Wq = np.asarray(Wq, dtype=np.float32)
    Wk = np.asarray(Wk, dtype=np.float32)
    Wv = np.asarray(Wv, dtype=np.float32)
    Wo = np.asarray(Wo, dtype=np.float32)
    bo = np.asarray(bo, dtype=np.float32)

    bs, S, d = x.shape
    nc = _get_nc(S)
    in_maps = make_in_maps(x, Wq, Wk, Wv, Wo)

    res = run_bass_kernel_spmd(nc, in_maps, core_ids=list(range(8)))
    outp = np.empty((bs, S, d), dtype=np.float32)
    for b in range(bs):
        outp[b] = res.results[2 * b]["out"] + res.results[2 * b + 1]["out"] + bo
    return outp
